# revision 7
# baseline (speedup 1.0000x reference)
"""Causal single-head attention kernel for TRN2 (one batch element per core).

Computes: out = softmax(causal((X_q Wq + bq)(X_k Wk + bk)^T / sqrt(H))) (X_v Wv + bv)
Shapes per core: Q,K,V [S, E]; Wq/Wk/Wv [E, H]; bq/bk/bv [H]; out [S, H].

v3 design notes:
- Input DMAs (SWDGE fp32->bf16 cast) issued first in round order Q_r, K_r,
  V_r (single SWDGE queue drains FIFO, so arrival order == consumption
  order); constants built on gpsimd between the first issues.
- Weights via HWDGE (parallel sync queue) as fp32 + on-chip bf16 cast;
  biases as single-descriptor [1,H] rows, transposed to [H,1] by a 1-row
  PE matmul (a [H,1]-strided DMA would emit 128 4-byte descriptors and
  stall the SDMA rings for ~10us).
- Packed-pair transposes: the bf16 input tile is bitcast to fp32 so each
  128x128 PE transpose moves *256* bf16 e-columns (transpose-mode is an
  exact bit mover). LDWEIGHTS (128 cols @1.2GHz, the transpose bottleneck)
  is paid once per 256 columns instead of 128 -> ~21us instead of ~44us.
  The projection then contracts even/odd e rows separately via stride-2
  bf16 APs against parity-split weight tiles.
- Causal diagonal blocks: column-sliced matmuls/exp (N=512..128); the one
  true-diagonal subtile is masked in-place by gpsimd affine_select.
- Attention chunk 3 split so j=0..11 runs while K3/V3 stream.
- Output transposed back in bf16, scaled by 1/rowsum, stored via HWDGE.
"""

from contextlib import ExitStack

import numpy as np

import concourse.bacc as bacc
import concourse.bass as bass
import concourse.mybir as mybir
import concourse.tile as tile
from concourse.masks import make_identity

F32 = mybir.dt.float32
BF16 = mybir.dt.bfloat16

CH = 512          # Sq chunk width (psum bank)
PT = 128          # partition tile


def build(S=2048, E=1024, H=128, n_cores=8):
    EC = E // PT              # E tiles (8)
    EB = E // (2 * PT)        # packed-pair e-blocks (4), 256 e-cols each
    NCH = S // CH             # Sq chunks (4)
    TPC = CH // PT            # S-tiles per chunk (4)
    scale = float(H) ** -0.5

    nc = bacc.Bacc("TRN2", target_bir_lowering=False, debug=False,
                   num_devices=n_cores)

    Qd = nc.declare_dram_parameter("Q", [S, E], F32, isOutput=False)
    Kd = nc.declare_dram_parameter("K", [S, E], F32, isOutput=False)
    Vd = nc.declare_dram_parameter("V", [S, E], F32, isOutput=False)
    Wqd = nc.declare_dram_parameter("Wq", [E, H], F32, isOutput=False)
    Wkd = nc.declare_dram_parameter("Wk", [E, H], F32, isOutput=False)
    Wvd = nc.declare_dram_parameter("Wv", [E, H], F32, isOutput=False)
    bqd = nc.declare_dram_parameter("bq", [H], F32, isOutput=False)
    bkd = nc.declare_dram_parameter("bk", [H], F32, isOutput=False)
    bvd = nc.declare_dram_parameter("bv", [H], F32, isOutput=False)
    outd = nc.declare_dram_parameter("out", [S, H], F32, isOutput=True)

    with tile.TileContext(nc) as tc, ExitStack() as ctx:
        persist = ctx.enter_context(tc.tile_pool(name="persist", bufs=1))
        stag_p = ctx.enter_context(tc.tile_pool(name="stag", bufs=3))
        xn_p = ctx.enter_context(tc.tile_pool(name="xn", bufs=12))
        xt_p = ctx.enter_context(tc.tile_pool(name="xt", bufs=8))
        ew_p = ctx.enter_context(tc.tile_pool(name="ew", bufs=8))
        small_p = ctx.enter_context(tc.tile_pool(name="small", bufs=4))

        ps_tp = ctx.enter_context(tc.tile_pool(name="ps_tp", bufs=2, space="PSUM"))
        ps_pj = ctx.enter_context(tc.tile_pool(name="ps_pj", bufs=1, space="PSUM"))
        ps_sc = ctx.enter_context(tc.tile_pool(name="ps_sc", bufs=3, space="PSUM"))
        ps_oT = ctx.enter_context(tc.tile_pool(name="ps_oT", bufs=1, space="PSUM"))
        ps_sm = ctx.enter_context(tc.tile_pool(name="ps_sm", bufs=1, space="PSUM"))

        # ---- bias rows on the HWDGE (sync) queue (single 512B descriptors;
        # their [H,1] fix-up is a 1-row PE matmul). Weights go on the gpsimd
        # SWDGE ring *ahead of the inputs* as fp32->bf16 cast loads, so they
        # arrive by ~14us and nothing downstream waits on a cast. ----
        brow = {}
        bsrc = {"q": bqd, "k": bkd, "v": bvd}
        for nm in ("q", "k", "v"):
            br = stag_p.tile([1, H], F32, tag=f"brow{nm}")
            nc.sync.dma_start(out=br, in_=bsrc[nm][:].unsqueeze(0))
            brow[nm] = br
        wts = {}
        wsrc = {"q": Wqd, "k": Wkd, "v": Wvd}
        for nm in ("q", "k", "v"):
            w = persist.tile([PT, EB, 2, H], BF16, tag=f"w{nm}")
            # weight row e = eb*256 + 2*p + par  ->  [p, eb, par, h]
            nc.gpsimd.dma_start(
                out=w,
                in_=wsrc[nm][:].rearrange("(c p two) h -> p c two h",
                                          p=PT, two=2))
            wts[nm] = w

        # ---- input DMAs (SWDGE cast loads) in stream order; constants
        # interleaved so gpsimd issues the first loads immediately ----
        xsrc = {"q": Qd, "k": Kd, "v": Vd}
        order = [(inp, r) for r in range(NCH) for inp in ("q", "k", "v")]
        xn = {}
        ident_f = ident_b = ones_col = one_1 = None
        for idx, (inp, r) in enumerate(order):
            t = xn_p.tile([PT, TPC, E], BF16, tag="xn")
            nc.gpsimd.dma_start(
                out=t, in_=xsrc[inp][r * CH:(r + 1) * CH, :].rearrange(
                    "(t p) e -> p t e", p=PT))
            xn[(inp, r)] = t
            if idx == 0:
                one_1 = persist.tile([1, 1], F32, tag="one_1")
                nc.gpsimd.memset(one_1, 1.0)
                ones_col = persist.tile([PT, 1], BF16, tag="ones_col")
                nc.gpsimd.memset(ones_col, 1.0)
            if idx == 1:
                ident_f = persist.tile([PT, PT], F32, tag="ident_f")
                make_identity(nc, ident_f)
            if idx == 2:
                ident_b = persist.tile([PT, PT], BF16, tag="ident_b")
                make_identity(nc, ident_b)

        # ---- bias fix-up ([1,H] -> [H,1]) and weight bf16 casts ----
        bias = {}
        for i, nm in enumerate(("q", "k", "v")):
            bps = ps_sc.tile([PT, CH], F32, tag="sc")
            nc.tensor.matmul(bps[:, 0:1], brow[nm][0:1, :], one_1[:],
                             start=True, stop=True)
            b = persist.tile([H, 1], F32, tag=f"b{nm}")
            nc.vector.tensor_copy(b, bps[:, 0:1])
            bias[nm] = b

        # persistent projected tensors
        qT = [persist.tile([H, CH], BF16, tag=f"qT{c}", name=f"qT{c}")
              for c in range(NCH)]
        kT = [persist.tile([H, CH], BF16, tag=f"kT{c}", name=f"kT{c}")
              for c in range(NCH)]
        vnat = persist.tile([PT, S // PT, H], BF16, tag="vnat")

        cnt = [0]

        def prep(inp, r):
            """Packed transpose of one X chunk + projection."""
            xnt = xn[(inp, r)]
            xf = xnt[:].bitcast(F32)          # [PT, TPC, E//2]
            xts = []
            for eb in range(EB):
                tp = ps_tp.tile([PT, TPC, PT], F32, tag="tp")
                for t in range(TPC):
                    nc.tensor.transpose(out=tp[:, t, :],
                                        in_=xf[:, t, eb * PT:(eb + 1) * PT],
                                        identity=ident_f[:])
                xt = xt_p.tile([PT, TPC, 2 * PT], BF16, tag="xt")
                # 3:1 DVE:ACT split (ACT also owns the exp stream)
                if cnt[0] % 4 == 3:
                    nc.scalar.copy(out=xt, in_=tp[:].bitcast(BF16))
                else:
                    nc.vector.tensor_copy(xt, tp[:].bitcast(BF16))
                cnt[0] += 1
                xts.append(xt)
            pj = ps_pj.tile([H, CH], F32, tag="pj")
            w = wts[inp]
            k = 0
            for eb in range(EB):
                xv = xts[eb][:].rearrange("p t (s two) -> p t s two", two=2)
                for par in range(2):
                    nc.tensor.matmul(pj, w[:, eb, par, :], xv[:, :, :, par],
                                     start=(k == 0), stop=(k == 2 * EB - 1))
                    k += 1
            if inp == "q":
                nc.vector.tensor_scalar_add(qT[r], pj, bias["q"][:])
            elif inp == "k":
                nc.vector.tensor_scalar_add(kT[r], pj, bias["k"][:])
            else:
                vT = small_p.tile([H, CH], BF16, tag="vT")
                nc.vector.tensor_scalar_add(vT, pj, bias["v"][:])
                vtp = ps_tp.tile([PT, TPC, PT], F32, tag="tp")
                vtb = vtp[:].bitcast(BF16)    # [PT, TPC, 2*PT]
                for t in range(TPC):
                    nc.tensor.transpose(out=vtb[:, t, 0:PT],
                                        in_=vT[:, t * PT:(t + 1) * PT],
                                        identity=ident_b[:])
                nc.vector.tensor_copy(
                    vnat[:, r * TPC:(r + 1) * TPC, :], vtb[:, :, 0:PT])

        def attn_piece(c, j0, j1, oT, sums):
            nk = (c + 1) * TPC

            def score_exp(j):
                kc, kt = divmod(j, TPC)
                off = kt * PT if kc == c else 0
                wp = ps_sc.tile([PT, CH], F32, tag="sc")
                nc.tensor.matmul(wp[:, off:CH], kT[kc][:, kt * PT:(kt + 1) * PT],
                                 qT[c][:, off:CH], start=True, stop=True)
                ew = ew_p.tile([PT, CH], BF16, tag="ew")
                nc.scalar.activation(out=ew[:, off:CH], in_=wp[:, off:CH],
                                     func=mybir.ActivationFunctionType.Exp,
                                     scale=scale)
                if kc == c:
                    # zero strictly-upper triangle of the diagonal subtile
                    nc.gpsimd.affine_select(
                        out=ew[:, off:off + PT], in_=ew[:, off:off + PT],
                        compare_op=mybir.AluOpType.is_ge, fill=0.0,
                        base=0, pattern=[[1, PT]], channel_multiplier=-1)
                return ew, off

            def sums_pv(j, ew, off):
                nc.tensor.matmul(sums[0:1, off:CH], ones_col[:], ew[:, off:CH],
                                 start=(j == 0), stop=(j == nk - 1))
                nc.tensor.matmul(oT[:, off:CH], vnat[:, j, :], ew[:, off:CH],
                                 start=(j == 0), stop=(j == nk - 1))

            # software pipeline: score/exp of j+1 issue ahead of sums/PV of j
            prev = None
            for j in range(j0, j1):
                cur = (j, *score_exp(j))
                if prev is not None:
                    sums_pv(*prev)
                prev = cur
            if prev is not None:
                sums_pv(*prev)

        def out_chunk(c, oT, sums):
            sums_sb = small_p.tile([1, CH], F32, tag="sums_sb")
            nc.vector.tensor_copy(sums_sb, sums[0:1, :])
            sumsT = ps_sc.tile([PT, CH], F32, tag="sc")
            for t in range(TPC):
                nc.tensor.matmul(sumsT[:, t:t + 1],
                                 sums_sb[0:1, t * PT:(t + 1) * PT],
                                 one_1[:], start=True, stop=True)
            recip = small_p.tile([PT, TPC], F32, tag="recip")
            nc.vector.reciprocal(recip, sumsT[:, 0:TPC])
            oT_sb = small_p.tile([H, CH], BF16, tag="oT_sb")
            nc.scalar.copy(out=oT_sb, in_=oT)
            otp = ps_tp.tile([PT, TPC, PT], F32, tag="tp")
            otb = otp[:].bitcast(BF16)
            for t in range(TPC):
                nc.tensor.transpose(out=otb[:, t, 0:PT],
                                    in_=oT_sb[:, t * PT:(t + 1) * PT],
                                    identity=ident_b[:])
            for t in range(TPC):
                ob = small_p.tile([PT, H], F32, tag="ob")
                nc.vector.tensor_scalar_mul(ob, otb[:, t, 0:PT],
                                            recip[:, t:t + 1])
                nc.sync.dma_start(
                    out=outd[c * CH + t * PT: c * CH + (t + 1) * PT, :], in_=ob)

        for c in range(NCH):
            prep("q", c)
            oT = ps_oT.tile([H, CH], F32, tag="oT")
            sums = ps_sm.tile([1, CH], F32, tag="sums")
            if c < NCH - 1:
                prep("k", c)
                prep("v", c)
                attn_piece(c, 0, (c + 1) * TPC, oT, sums)
            else:
                # run j tiles that only need earlier rounds while K3/V3 stream
                attn_piece(c, 0, c * TPC, oT, sums)
                prep("k", c)
                prep("v", c)
                attn_piece(c, c * TPC, (c + 1) * TPC, oT, sums)
            out_chunk(c, oT, sums)

    nc.compile()
    return nc


_NC_CACHE = {}


def _get_nc():
    if "nc" not in _NC_CACHE:
        _NC_CACHE["nc"] = build(S=2048, E=1024, H=128, n_cores=8)
    return _NC_CACHE["nc"]


def kernel(Q, K, V, mask=None, Wq=None, bq=None, Wk=None, bk=None,
           Wv=None, bv=None, **_):
    """Full-input entry point: Q/K/V [8, 2048, 1024] fp32 -> out [8, 2048, 128].

    Data-parallel over batch: core i computes batch element i. The causal
    mask input is ignored (causality is hardcoded in the kernel structure).
    """
    from concourse.bass_utils import run_bass_kernel_spmd

    B = Q.shape[0]
    nc = _get_nc()
    f32 = np.float32
    in_maps = []
    for i in range(B):
        in_maps.append({
            "Q": np.ascontiguousarray(Q[i], dtype=f32),
            "K": np.ascontiguousarray(K[i], dtype=f32),
            "V": np.ascontiguousarray(V[i], dtype=f32),
            "Wq": np.ascontiguousarray(Wq, dtype=f32),
            "Wk": np.ascontiguousarray(Wk, dtype=f32),
            "Wv": np.ascontiguousarray(Wv, dtype=f32),
            "bq": np.ascontiguousarray(bq, dtype=f32),
            "bk": np.ascontiguousarray(bk, dtype=f32),
            "bv": np.ascontiguousarray(bv, dtype=f32),
        })
    r = run_bass_kernel_spmd(nc, in_maps, list(range(B)))
    return np.stack([r.results[i]["out"] for i in range(B)]).astype(np.float32)


# revision 9
# speedup vs baseline: 1.1546x; 1.1546x over previous
"""Causal single-head attention kernel for TRN2 (one batch element per core).

Computes: out = softmax(causal((X_q Wq + bq)(X_k Wk + bk)^T / sqrt(H))) (X_v Wv + bv)
Shapes per core: Q,K,V [S, E]; Wq/Wk/Wv [E, H]; bq/bk/bv [H]; out [S, H].

v3 design notes:
- Input DMAs (SWDGE fp32->bf16 cast) issued first in round order Q_r, K_r,
  V_r (single SWDGE queue drains FIFO, so arrival order == consumption
  order); constants built on gpsimd between the first issues.
- Weights via HWDGE (parallel sync queue) as fp32 + on-chip bf16 cast;
  biases as single-descriptor [1,H] rows, transposed to [H,1] by a 1-row
  PE matmul (a [H,1]-strided DMA would emit 128 4-byte descriptors and
  stall the SDMA rings for ~10us).
- Packed-pair transposes: the bf16 input tile is bitcast to fp32 so each
  128x128 PE transpose moves *256* bf16 e-columns (transpose-mode is an
  exact bit mover). LDWEIGHTS (128 cols @1.2GHz, the transpose bottleneck)
  is paid once per 256 columns instead of 128 -> ~21us instead of ~44us.
  The projection then contracts even/odd e rows separately via stride-2
  bf16 APs against parity-split weight tiles.
- Causal diagonal blocks: column-sliced matmuls/exp (N=512..128); the one
  true-diagonal subtile is masked in-place by gpsimd affine_select.
- Attention chunk 3 split so j=0..11 runs while K3/V3 stream.
- Output transposed back in bf16, scaled by 1/rowsum, stored via HWDGE.
"""

from contextlib import ExitStack

import numpy as np

import concourse.bacc as bacc
import concourse.bass as bass
import concourse.mybir as mybir
import concourse.tile as tile
from concourse.masks import make_identity

F32 = mybir.dt.float32
BF16 = mybir.dt.bfloat16

CH = 512          # Sq chunk width (psum bank)
PT = 128          # partition tile


def build(S=2048, E=1024, H=128, n_cores=8):
    EC = E // PT              # E tiles (8)
    EB = E // (2 * PT)        # packed-pair e-blocks (4), 256 e-cols each
    NCH = S // CH             # Sq chunks (4)
    TPC = CH // PT            # S-tiles per chunk (4)
    scale = float(H) ** -0.5

    nc = bacc.Bacc("TRN2", target_bir_lowering=False, debug=False,
                   num_devices=n_cores)

    Qd = nc.declare_dram_parameter("Q", [S, E], F32, isOutput=False)
    Kd = nc.declare_dram_parameter("K", [S, E], F32, isOutput=False)
    Vd = nc.declare_dram_parameter("V", [S, E], F32, isOutput=False)
    Wqd = nc.declare_dram_parameter("Wq", [E, H], F32, isOutput=False)
    Wkd = nc.declare_dram_parameter("Wk", [E, H], F32, isOutput=False)
    Wvd = nc.declare_dram_parameter("Wv", [E, H], F32, isOutput=False)
    bqd = nc.declare_dram_parameter("bq", [H], F32, isOutput=False)
    bkd = nc.declare_dram_parameter("bk", [H], F32, isOutput=False)
    bvd = nc.declare_dram_parameter("bv", [H], F32, isOutput=False)
    outd = nc.declare_dram_parameter("out", [S, H], F32, isOutput=True)

    with tile.TileContext(nc) as tc, ExitStack() as ctx:
        persist = ctx.enter_context(tc.tile_pool(name="persist", bufs=1))
        stag_p = ctx.enter_context(tc.tile_pool(name="stag", bufs=3))
        xn_p = ctx.enter_context(tc.tile_pool(name="xn", bufs=12))
        xt_p = ctx.enter_context(tc.tile_pool(name="xt", bufs=8))
        ew_p = ctx.enter_context(tc.tile_pool(name="ew", bufs=8))
        small_p = ctx.enter_context(tc.tile_pool(name="small", bufs=4))

        ps_tp = ctx.enter_context(tc.tile_pool(name="ps_tp", bufs=2, space="PSUM"))
        ps_pj = ctx.enter_context(tc.tile_pool(name="ps_pj", bufs=1, space="PSUM"))
        ps_sc = ctx.enter_context(tc.tile_pool(name="ps_sc", bufs=3, space="PSUM"))
        ps_oT = ctx.enter_context(tc.tile_pool(name="ps_oT", bufs=1, space="PSUM"))
        ps_sm = ctx.enter_context(tc.tile_pool(name="ps_sm", bufs=1, space="PSUM"))

        # ---- bias rows on the HWDGE (sync) queue (single 512B descriptors;
        # their [H,1] fix-up is a 1-row PE matmul). Weights go on the gpsimd
        # SWDGE ring *ahead of the inputs* as fp32->bf16 cast loads, so they
        # arrive by ~14us and nothing downstream waits on a cast. ----
        brow = {}
        bsrc = {"q": bqd, "k": bkd, "v": bvd}
        for nm in ("q", "k", "v"):
            br = stag_p.tile([1, H], F32, tag=f"brow{nm}")
            nc.sync.dma_start(out=br, in_=bsrc[nm][:].unsqueeze(0))
            brow[nm] = br
        wts = {}
        wsrc = {"q": Wqd, "k": Wkd, "v": Wvd}
        for nm in ("q", "k", "v"):
            w = persist.tile([PT, EB, 2, H], BF16, tag=f"w{nm}")
            # weight row e = eb*256 + 2*p + par  ->  [p, eb, par, h]
            nc.gpsimd.dma_start(
                out=w,
                in_=wsrc[nm][:].rearrange("(c p two) h -> p c two h",
                                          p=PT, two=2))
            wts[nm] = w

        # ---- input DMAs (SWDGE cast loads) in stream order; constants
        # interleaved so gpsimd issues the first loads immediately ----
        xsrc = {"q": Qd, "k": Kd, "v": Vd}
        order = [(inp, r) for r in range(NCH) for inp in ("q", "k", "v")]
        xn = {}
        ident_f = ident_b = ones_col = one_1 = None
        for idx, (inp, r) in enumerate(order):
            t = xn_p.tile([PT, TPC, E], BF16, tag="xn")
            nc.gpsimd.dma_start(
                out=t, in_=xsrc[inp][r * CH:(r + 1) * CH, :].rearrange(
                    "(t p) e -> p t e", p=PT))
            xn[(inp, r)] = t
            if idx == 0:
                one_1 = persist.tile([1, 1], F32, tag="one_1")
                nc.gpsimd.memset(one_1, 1.0)
                ones_col = persist.tile([PT, 1], BF16, tag="ones_col")
                nc.gpsimd.memset(ones_col, 1.0)
            if idx == 1:
                ident_f = persist.tile([PT, PT], F32, tag="ident_f")
                make_identity(nc, ident_f)
            if idx == 2:
                ident_b = persist.tile([PT, PT], BF16, tag="ident_b")
                make_identity(nc, ident_b)

        # ---- bias fix-up ([1,H] -> [H,1]) and weight bf16 casts ----
        bias = {}
        for i, nm in enumerate(("q", "k", "v")):
            bps = ps_sc.tile([PT, CH], F32, tag="sc")
            nc.tensor.matmul(bps[:, 0:1], brow[nm][0:1, :], one_1[:],
                             start=True, stop=True)
            b = persist.tile([H, 1], F32, tag=f"b{nm}")
            nc.vector.tensor_copy(b, bps[:, 0:1])
            bias[nm] = b

        # persistent projected tensors
        qT = [persist.tile([H, CH], BF16, tag=f"qT{c}", name=f"qT{c}")
              for c in range(NCH)]
        kT = [persist.tile([H, CH], BF16, tag=f"kT{c}", name=f"kT{c}")
              for c in range(NCH)]
        vnat = persist.tile([PT, S // PT, H], BF16, tag="vnat")

        cnt = [0]

        def prep(inp, r):
            """Packed transpose of one X chunk + projection."""
            xnt = xn[(inp, r)]
            xf = xnt[:].bitcast(F32)          # [PT, TPC, E//2]
            xts = []
            for eb in range(EB):
                tp = ps_tp.tile([PT, TPC, PT], F32, tag="tp")
                for t in range(TPC):
                    nc.tensor.transpose(out=tp[:, t, :],
                                        in_=xf[:, t, eb * PT:(eb + 1) * PT],
                                        identity=ident_f[:])
                xt = xt_p.tile([PT, TPC, 2 * PT], BF16, tag="xt")
                # 2:1 DVE:ACT split (ACT also owns the exp stream)
                if cnt[0] % 3 == 2:
                    nc.scalar.copy(out=xt, in_=tp[:].bitcast(BF16))
                else:
                    nc.vector.tensor_copy(xt, tp[:].bitcast(BF16))
                cnt[0] += 1
                xts.append(xt)
            pj = ps_pj.tile([H, CH], F32, tag="pj")
            w = wts[inp]
            k = 0
            for eb in range(EB):
                xv = xts[eb][:].rearrange("p t (s two) -> p t s two", two=2)
                for par in range(2):
                    nc.tensor.matmul(pj, w[:, eb, par, :], xv[:, :, :, par],
                                     start=(k == 0), stop=(k == 2 * EB - 1))
                    k += 1
            if inp == "q":
                nc.vector.tensor_scalar_add(qT[r], pj, bias["q"][:])
            elif inp == "k":
                nc.vector.tensor_scalar_add(kT[r], pj, bias["k"][:])
            else:
                vT = small_p.tile([H, CH], BF16, tag="vT")
                nc.vector.tensor_scalar_add(vT, pj, bias["v"][:])
                vtp = ps_tp.tile([PT, TPC, PT], F32, tag="tp")
                vtb = vtp[:].bitcast(BF16)    # [PT, TPC, 2*PT]
                for t in range(TPC):
                    nc.tensor.transpose(out=vtb[:, t, 0:PT],
                                        in_=vT[:, t * PT:(t + 1) * PT],
                                        identity=ident_b[:])
                nc.vector.tensor_copy(
                    vnat[:, r * TPC:(r + 1) * TPC, :], vtb[:, :, 0:PT])

        def attn_piece(c, j0, j1, oT, sums, state):
            nk = (c + 1) * TPC
            # state: [sums_started, pending (j, ew) of an unpaired full tile]

            def score_exp(j):
                kc, kt = divmod(j, TPC)
                off = kt * PT if kc == c else 0
                wp = ps_sc.tile([PT, CH], F32, tag="sc")
                nc.tensor.matmul(wp[:, off:CH], kT[kc][:, kt * PT:(kt + 1) * PT],
                                 qT[c][:, off:CH], start=True, stop=True)
                ew = ew_p.tile([PT, CH], BF16, tag="ew")
                nc.scalar.activation(out=ew[:, off:CH], in_=wp[:, off:CH],
                                     func=mybir.ActivationFunctionType.Exp,
                                     scale=scale)
                if kc == c:
                    # zero strictly-upper triangle of the diagonal subtile
                    nc.gpsimd.affine_select(
                        out=ew[:, off:off + PT], in_=ew[:, off:off + PT],
                        compare_op=mybir.AluOpType.is_ge, fill=0.0,
                        base=0, pattern=[[1, PT]], channel_multiplier=-1)
                return ew, off

            def sums_mm(src, off, last):
                nc.tensor.matmul(sums[0:1, off:CH], ones_col[:], src[:, off:CH],
                                 start=not state[0], stop=last)
                state[0] = True

            def sums_pv(j, ew, off):
                diag = j >= c * TPC
                last = j == nk - 1
                if not diag and state[1] is None and j + 1 < c * TPC:
                    state[1] = (j, ew)        # wait for a partner
                elif not diag and state[1] is not None:
                    ewp = ew_p.tile([PT, CH], BF16, tag="ewp")
                    nc.vector.tensor_add(ewp, state[1][1], ew)
                    state[1] = None
                    sums_mm(ewp, 0, last)
                else:
                    sums_mm(ew, off, last)
                nc.tensor.matmul(oT[:, off:CH], vnat[:, j, :], ew[:, off:CH],
                                 start=(j == 0), stop=last)

            # software pipeline, depth 2: score/exp run two j ahead of sums/PV
            from collections import deque
            q = deque()
            for j in range(j0, j1):
                q.append((j, *score_exp(j)))
                if len(q) > 2:
                    sums_pv(*q.popleft())
            while q:
                sums_pv(*q.popleft())

        def out_chunk(c, oT, sums):
            sums_sb = small_p.tile([1, CH], F32, tag="sums_sb")
            nc.vector.tensor_copy(sums_sb, sums[0:1, :])
            sumsT = ps_sc.tile([PT, CH], F32, tag="sc")
            for t in range(TPC):
                nc.tensor.matmul(sumsT[:, t:t + 1],
                                 sums_sb[0:1, t * PT:(t + 1) * PT],
                                 one_1[:], start=True, stop=True)
            recip = small_p.tile([PT, TPC], F32, tag="recip")
            nc.vector.reciprocal(recip, sumsT[:, 0:TPC])
            oT_sb = small_p.tile([H, CH], BF16, tag="oT_sb")
            nc.vector.tensor_copy(oT_sb, oT)
            otp = ps_tp.tile([PT, TPC, PT], F32, tag="tp")
            otb = otp[:].bitcast(BF16)
            for t in range(TPC):
                nc.tensor.transpose(out=otb[:, t, 0:PT],
                                    in_=oT_sb[:, t * PT:(t + 1) * PT],
                                    identity=ident_b[:])
            for t in range(TPC):
                ob = small_p.tile([PT, H], F32, tag="ob")
                nc.vector.tensor_scalar_mul(ob, otb[:, t, 0:PT],
                                            recip[:, t:t + 1])
                nc.sync.dma_start(
                    out=outd[c * CH + t * PT: c * CH + (t + 1) * PT, :], in_=ob)

        for c in range(NCH):
            prep("q", c)
            oT = ps_oT.tile([H, CH], F32, tag="oT")
            sums = ps_sm.tile([1, CH], F32, tag="sums")
            state = [False, None]
            if c < NCH - 1:
                prep("k", c)
                prep("v", c)
                attn_piece(c, 0, (c + 1) * TPC, oT, sums, state)
            else:
                # run j tiles that only need earlier rounds while K3/V3 stream
                attn_piece(c, 0, c * TPC, oT, sums, state)
                prep("k", c)
                prep("v", c)
                attn_piece(c, c * TPC, (c + 1) * TPC, oT, sums, state)
            out_chunk(c, oT, sums)

    nc.compile()
    return nc


_NC_CACHE = {}


def _get_nc():
    if "nc" not in _NC_CACHE:
        _NC_CACHE["nc"] = build(S=2048, E=1024, H=128, n_cores=8)
    return _NC_CACHE["nc"]


def kernel(Q, K, V, mask=None, Wq=None, bq=None, Wk=None, bk=None,
           Wv=None, bv=None, **_):
    """Full-input entry point: Q/K/V [8, 2048, 1024] fp32 -> out [8, 2048, 128].

    Data-parallel over batch: core i computes batch element i. The causal
    mask input is ignored (causality is hardcoded in the kernel structure).
    """
    from concourse.bass_utils import run_bass_kernel_spmd

    B = Q.shape[0]
    nc = _get_nc()
    f32 = np.float32
    in_maps = []
    for i in range(B):
        in_maps.append({
            "Q": np.ascontiguousarray(Q[i], dtype=f32),
            "K": np.ascontiguousarray(K[i], dtype=f32),
            "V": np.ascontiguousarray(V[i], dtype=f32),
            "Wq": np.ascontiguousarray(Wq, dtype=f32),
            "Wk": np.ascontiguousarray(Wk, dtype=f32),
            "Wv": np.ascontiguousarray(Wv, dtype=f32),
            "bq": np.ascontiguousarray(bq, dtype=f32),
            "bk": np.ascontiguousarray(bk, dtype=f32),
            "bv": np.ascontiguousarray(bv, dtype=f32),
        })
    r = run_bass_kernel_spmd(nc, in_maps, list(range(B)))
    return np.stack([r.results[i]["out"] for i in range(B)]).astype(np.float32)


# revision 11
# speedup vs baseline: 1.3581x; 1.1762x over previous
"""Causal single-head attention kernel for TRN2 (one batch element per core).

Computes: out = softmax(causal((X_q Wq + bq)(X_k Wk + bk)^T / sqrt(H))) (X_v Wv + bv)
Shapes per core: Q,K,V [S, E]; Wq/Wk/Wv [E, H]; bq/bk/bv [H]; out [S, H].

v3 design notes:
- Input DMAs (SWDGE fp32->bf16 cast) issued first in round order Q_r, K_r,
  V_r (single SWDGE queue drains FIFO, so arrival order == consumption
  order); constants built on gpsimd between the first issues.
- Weights via HWDGE (parallel sync queue) as fp32 + on-chip bf16 cast;
  biases as single-descriptor [1,H] rows, transposed to [H,1] by a 1-row
  PE matmul (a [H,1]-strided DMA would emit 128 4-byte descriptors and
  stall the SDMA rings for ~10us).
- Packed-pair transposes: the bf16 input tile is bitcast to fp32 so each
  128x128 PE transpose moves *256* bf16 e-columns (transpose-mode is an
  exact bit mover). LDWEIGHTS (128 cols @1.2GHz, the transpose bottleneck)
  is paid once per 256 columns instead of 128 -> ~21us instead of ~44us.
  The projection then contracts even/odd e rows separately via stride-2
  bf16 APs against parity-split weight tiles.
- Causal diagonal blocks: column-sliced matmuls/exp (N=512..128); the one
  true-diagonal subtile is masked in-place by gpsimd affine_select.
- Attention chunk 3 split so j=0..11 runs while K3/V3 stream.
- Output transposed back in bf16, scaled by 1/rowsum, stored via HWDGE.
"""

from contextlib import ExitStack

import numpy as np

import concourse.bacc as bacc
import concourse.bass as bass
import concourse.mybir as mybir
import concourse.tile as tile
from concourse.masks import make_identity

F32 = mybir.dt.float32
BF16 = mybir.dt.bfloat16

CH = 512          # Sq chunk width (psum bank)
PT = 128          # partition tile


def build(S=2048, E=1024, H=128, n_cores=8):
    EC = E // PT              # E tiles (8)
    EB = E // (2 * PT)        # packed-pair e-blocks (4), 256 e-cols each
    NCH = S // CH             # Sq chunks (4)
    TPC = CH // PT            # S-tiles per chunk (4)
    scale = float(H) ** -0.5

    nc = bacc.Bacc("TRN2", target_bir_lowering=False, debug=False,
                   num_devices=n_cores)

    Qd = nc.declare_dram_parameter("Q", [S, E], F32, isOutput=False)
    Kd = nc.declare_dram_parameter("K", [S, E], F32, isOutput=False)
    Vd = nc.declare_dram_parameter("V", [S, E], F32, isOutput=False)
    Wqd = nc.declare_dram_parameter("Wq", [E, H], F32, isOutput=False)
    Wkd = nc.declare_dram_parameter("Wk", [E, H], F32, isOutput=False)
    Wvd = nc.declare_dram_parameter("Wv", [E, H], F32, isOutput=False)
    bqd = nc.declare_dram_parameter("bq", [H], F32, isOutput=False)
    bkd = nc.declare_dram_parameter("bk", [H], F32, isOutput=False)
    bvd = nc.declare_dram_parameter("bv", [H], F32, isOutput=False)
    outd = nc.declare_dram_parameter("out", [S, H], F32, isOutput=True)

    with tile.TileContext(nc) as tc, ExitStack() as ctx:
        persist = ctx.enter_context(tc.tile_pool(name="persist", bufs=1))
        stag_p = ctx.enter_context(tc.tile_pool(name="stag", bufs=3))
        xn_p = ctx.enter_context(tc.tile_pool(name="xn", bufs=12))
        xt_p = ctx.enter_context(tc.tile_pool(name="xt", bufs=8))
        ew_p = ctx.enter_context(tc.tile_pool(name="ew", bufs=8))
        small_p = ctx.enter_context(tc.tile_pool(name="small", bufs=4))

        ps_tp = ctx.enter_context(tc.tile_pool(name="ps_tp", bufs=2, space="PSUM"))
        ps_pj = ctx.enter_context(tc.tile_pool(name="ps_pj", bufs=1, space="PSUM"))
        ps_sc = ctx.enter_context(tc.tile_pool(name="ps_sc", bufs=3, space="PSUM"))
        ps_oT = ctx.enter_context(tc.tile_pool(name="ps_oT", bufs=1, space="PSUM"))
        ps_sm = ctx.enter_context(tc.tile_pool(name="ps_sm", bufs=1, space="PSUM"))

        # ---- bias rows on the HWDGE (sync) queue (single 512B descriptors;
        # their [H,1] fix-up is a 1-row PE matmul). Weights go on the gpsimd
        # SWDGE ring *ahead of the inputs* as fp32->bf16 cast loads, so they
        # arrive by ~14us and nothing downstream waits on a cast. ----
        brow = {}
        bsrc = {"q": bqd, "k": bkd, "v": bvd}
        for nm in ("q", "k", "v"):
            br = stag_p.tile([1, H], F32, tag=f"brow{nm}")
            nc.sync.dma_start(out=br, in_=bsrc[nm][:].unsqueeze(0))
            brow[nm] = br
        wts = {}
        wsrc = {"q": Wqd, "k": Wkd, "v": Wvd}

        def load_w(nm):
            w = persist.tile([PT, EB, 2, H], BF16, tag=f"w{nm}",
                             name=f"w{nm}")
            # weight row e = eb*256 + 2*p + par  ->  [p, eb, par, h]
            nc.gpsimd.dma_start(
                out=w,
                in_=wsrc[nm][:].rearrange("(c p two) h -> p c two h",
                                          p=PT, two=2))
            wts[nm] = w

        # ---- input DMAs (SWDGE cast loads) in stream order; constants
        # interleaved so gpsimd issues the first loads immediately ----
        xsrc = {"q": Qd, "k": Kd, "v": Vd}
        order = [(inp, r) for r in range(NCH) for inp in ("q", "k", "v")]
        xn = {}
        ident_f = ident_b = ones_col = one_1 = None
        for idx, (inp, r) in enumerate(order):
            if idx < 3:
                load_w(inp)       # weight DMA right before its first user
            t = xn_p.tile([PT, TPC, E], BF16, tag="xn")
            nc.gpsimd.dma_start(
                out=t, in_=xsrc[inp][r * CH:(r + 1) * CH, :].rearrange(
                    "(t p) e -> p t e", p=PT))
            xn[(inp, r)] = t
            if idx == 0:
                one_1 = persist.tile([1, 1], F32, tag="one_1")
                nc.gpsimd.memset(one_1, 1.0)
                ones_col = persist.tile([PT, 1], BF16, tag="ones_col")
                nc.gpsimd.memset(ones_col, 1.0)
            if idx == 1:
                ident_f = persist.tile([PT, PT], F32, tag="ident_f")
                make_identity(nc, ident_f)
            if idx == 2:
                ident_b = persist.tile([PT, PT], BF16, tag="ident_b")
                make_identity(nc, ident_b)

        # ---- bias fix-up ([1,H] -> [H,1]) and weight bf16 casts ----
        bias = {}
        for i, nm in enumerate(("q", "k", "v")):
            bps = ps_sc.tile([PT, CH], F32, tag="sc")
            nc.tensor.matmul(bps[:, 0:1], brow[nm][0:1, :], one_1[:],
                             start=True, stop=True)
            b = persist.tile([H, 1], F32, tag=f"b{nm}")
            nc.vector.tensor_copy(b, bps[:, 0:1])
            bias[nm] = b

        # persistent projected tensors
        qT = [persist.tile([H, CH], BF16, tag=f"qT{c}", name=f"qT{c}")
              for c in range(NCH)]
        kT = [persist.tile([H, CH], BF16, tag=f"kT{c}", name=f"kT{c}")
              for c in range(NCH)]
        vnat = persist.tile([PT, S // PT, H], BF16, tag="vnat")

        cnt = [0]

        def prep(inp, r):
            """Packed transpose of one X chunk + projection."""
            xnt = xn[(inp, r)]
            xf = xnt[:].bitcast(F32)          # [PT, TPC, E//2]
            xts = []
            for eb in range(EB):
                tp = ps_tp.tile([PT, TPC, PT], F32, tag="tp")
                for t in range(TPC):
                    nc.tensor.transpose(out=tp[:, t, :],
                                        in_=xf[:, t, eb * PT:(eb + 1) * PT],
                                        identity=ident_f[:])
                xt = xt_p.tile([PT, TPC, 2 * PT], BF16, tag="xt")
                # 2:1 DVE:ACT split (ACT also owns the exp stream)
                if cnt[0] % 3 == 2:
                    nc.scalar.copy(out=xt, in_=tp[:].bitcast(BF16))
                else:
                    nc.vector.tensor_copy(xt, tp[:].bitcast(BF16))
                cnt[0] += 1
                xts.append(xt)
            pj = ps_pj.tile([H, CH], F32, tag="pj")
            w = wts[inp]
            k = 0
            for eb in range(EB):
                xv = xts[eb][:].rearrange("p t (s two) -> p t s two", two=2)
                for par in range(2):
                    nc.tensor.matmul(pj, w[:, eb, par, :], xv[:, :, :, par],
                                     start=(k == 0), stop=(k == 2 * EB - 1))
                    k += 1
            if inp == "q":
                nc.vector.tensor_scalar_add(qT[r], pj, bias["q"][:])
            elif inp == "k":
                nc.vector.tensor_scalar_add(kT[r], pj, bias["k"][:])
            else:
                vT = small_p.tile([H, CH], BF16, tag="vT")
                nc.vector.tensor_scalar_add(vT, pj, bias["v"][:])
                vtp = ps_tp.tile([PT, TPC, PT], F32, tag="tp")
                vtb = vtp[:].bitcast(BF16)    # [PT, TPC, 2*PT]
                for t in range(TPC):
                    nc.tensor.transpose(out=vtb[:, t, 0:PT],
                                        in_=vT[:, t * PT:(t + 1) * PT],
                                        identity=ident_b[:])
                nc.vector.tensor_copy(
                    vnat[:, r * TPC:(r + 1) * TPC, :], vtb[:, :, 0:PT])

        def attn_piece(c, j0, j1, oT, sums, state):
            nk = (c + 1) * TPC
            # state: [sums_started, pending (j, ew) of an unpaired full tile]

            def score_exp(j):
                kc, kt = divmod(j, TPC)
                off = kt * PT if kc == c else 0
                wp = ps_sc.tile([PT, CH], F32, tag="sc")
                nc.tensor.matmul(wp[:, off:CH], kT[kc][:, kt * PT:(kt + 1) * PT],
                                 qT[c][:, off:CH], start=True, stop=True)
                ew = ew_p.tile([PT, CH], BF16, tag="ew")
                nc.scalar.activation(out=ew[:, off:CH], in_=wp[:, off:CH],
                                     func=mybir.ActivationFunctionType.Exp,
                                     scale=scale)
                if kc == c:
                    # zero strictly-upper triangle of the diagonal subtile
                    nc.gpsimd.affine_select(
                        out=ew[:, off:off + PT], in_=ew[:, off:off + PT],
                        compare_op=mybir.AluOpType.is_ge, fill=0.0,
                        base=0, pattern=[[1, PT]], channel_multiplier=-1)
                return ew, off

            def sums_mm(src, off, last):
                nc.tensor.matmul(sums[0:1, off:CH], ones_col[:], src[:, off:CH],
                                 start=not state[0], stop=last)
                state[0] = True

            def sums_pv(j, ew, off):
                diag = j >= c * TPC
                last = j == nk - 1
                if not diag and state[1] is None and j + 1 < c * TPC:
                    state[1] = (j, ew)        # wait for a partner
                elif not diag and state[1] is not None:
                    ewp = ew_p.tile([PT, CH], BF16, tag="ewp")
                    nc.vector.tensor_add(ewp, state[1][1], ew)
                    state[1] = None
                    sums_mm(ewp, 0, last)
                else:
                    sums_mm(ew, off, last)
                nc.tensor.matmul(oT[:, off:CH], vnat[:, j, :], ew[:, off:CH],
                                 start=(j == 0), stop=last)

            # software pipeline, depth 2: score/exp run two j ahead of sums/PV
            from collections import deque
            q = deque()
            for j in range(j0, j1):
                q.append((j, *score_exp(j)))
                if len(q) > 2:
                    sums_pv(*q.popleft())
            while q:
                sums_pv(*q.popleft())

        def out_chunk(c, oT, sums):
            sums_sb = small_p.tile([1, CH], BF16, tag="sums_sb")
            nc.vector.tensor_copy(sums_sb, sums[0:1, :])
            sumsT = ps_sc.tile([PT, CH], F32, tag="sc")
            sumsTb = sumsT[:].bitcast(BF16)
            for t in range(TPC):
                nc.tensor.transpose(out=sumsTb[:, 2 * t:2 * t + 1],
                                    in_=sums_sb[0:1, t * PT:(t + 1) * PT],
                                    identity=ones_col[0:1, :])
            recip = small_p.tile([PT, TPC], F32, tag="recip")
            nc.vector.reciprocal(
                recip, sumsTb[:].rearrange("p (a b) -> p a b", b=2)[:, 0:TPC, 0])
            oT_sb = small_p.tile([H, CH], BF16, tag="oT_sb")
            nc.vector.tensor_copy(oT_sb, oT)
            otp = ps_sc.tile([PT, CH], F32, tag="sc")
            otb = otp[:].bitcast(BF16).rearrange("p (t x) -> p t x", x=2 * PT)
            for t in range(TPC):
                nc.tensor.transpose(out=otb[:, t, 0:PT],
                                    in_=oT_sb[:, t * PT:(t + 1) * PT],
                                    identity=ident_b[:])
            for t in range(TPC):
                ob = small_p.tile([PT, H], F32, tag="ob")
                nc.vector.tensor_scalar_mul(ob, otb[:, t, 0:PT],
                                            recip[:, t:t + 1])
                nc.sync.dma_start(
                    out=outd[c * CH + t * PT: c * CH + (t + 1) * PT, :], in_=ob)

        for c in range(NCH):
            prep("q", c)
            oT = ps_oT.tile([H, CH], F32, tag="oT")
            sums = ps_sm.tile([1, CH], F32, tag="sums")
            state = [False, None]
            if c < NCH - 1:
                prep("k", c)
                prep("v", c)
                attn_piece(c, 0, (c + 1) * TPC, oT, sums, state)
            else:
                # run j tiles that only need earlier rounds while K3/V3 stream
                attn_piece(c, 0, c * TPC, oT, sums, state)
                prep("k", c)
                prep("v", c)
                attn_piece(c, c * TPC, (c + 1) * TPC, oT, sums, state)
            out_chunk(c, oT, sums)

    nc.compile()
    return nc


_NC_CACHE = {}


def _get_nc():
    if "nc" not in _NC_CACHE:
        _NC_CACHE["nc"] = build(S=2048, E=1024, H=128, n_cores=8)
    return _NC_CACHE["nc"]


def kernel(Q, K, V, mask=None, Wq=None, bq=None, Wk=None, bk=None,
           Wv=None, bv=None, **_):
    """Full-input entry point: Q/K/V [8, 2048, 1024] fp32 -> out [8, 2048, 128].

    Data-parallel over batch: core i computes batch element i. The causal
    mask input is ignored (causality is hardcoded in the kernel structure).
    """
    from concourse.bass_utils import run_bass_kernel_spmd

    B = Q.shape[0]
    nc = _get_nc()
    f32 = np.float32
    in_maps = []
    for i in range(B):
        in_maps.append({
            "Q": np.ascontiguousarray(Q[i], dtype=f32),
            "K": np.ascontiguousarray(K[i], dtype=f32),
            "V": np.ascontiguousarray(V[i], dtype=f32),
            "Wq": np.ascontiguousarray(Wq, dtype=f32),
            "Wk": np.ascontiguousarray(Wk, dtype=f32),
            "Wv": np.ascontiguousarray(Wv, dtype=f32),
            "bq": np.ascontiguousarray(bq, dtype=f32),
            "bk": np.ascontiguousarray(bk, dtype=f32),
            "bv": np.ascontiguousarray(bv, dtype=f32),
        })
    r = run_bass_kernel_spmd(nc, in_maps, list(range(B)))
    return np.stack([r.results[i]["out"] for i in range(B)]).astype(np.float32)
